# revision 9
# baseline (speedup 1.0000x reference)
"""Trainium2 Bass kernel for the spectral-gating network (nn_DAPSO).

Model (B=4, C=256, H=W=256):
  - channels 0:128   : y_h = irfft(Gh * rfft(x, axis=H))   (per-channel gate)
  - channels 128:256 : y_w = irfft(Gw * rfft(x, axis=W))
  - gates Gh/Gw from tiny MLPs (computed on device)
  - channel attention: s = sigmoid(dw(gelu(W1 @ mean_hw(y) + b)))  -> y *= s
  - y2 = gelu(BN(lc_w @ y));  out = x + y2

Key algorithmic mapping: irfft(G*rfft(x)) along an axis of length N equals
T^T diag(ghat) T x with T the orthonormal real DFT basis (cos/sin rows), so
both branches become dense TensorE matmuls (no FFT).

Sharding: 8 cores = 4 batches x 2 w-halves. Each core computes BOTH branch
outputs for its (batch, w-half) spatial region: the H-branch needs only its
w-columns; the W-branch contracts the full W axis (its forward transform is
duplicated between the pair of cores). The only cross-core communication is
a 1KB AllReduce of the pooled channel means.

Channel attention pooling is computed analytically from input sums (DC
coefficient trick), so it never blocks on branch outputs.

v2 layout/throughput rework vs the original baseline (799us):
  - no f32 residual pre-copy / accumulate-DMA: the conv stage loads x as
    bf16 tiles, adds the residual on GpSimd, and writes a bf16 output
    (host upcasts). Saves ~96 MiB/core of DRAM traffic.
  - all external tensors host-pre-tiled so every DMA is contiguous per
    partition; the branch->conv scratch is c-major so the DMA-transpose
    cost sits on the (PE-bound) branch phase, not the (DMA-bound) conv
    phase.
  - matmul loops restructured for dense PE streams (HAM stays warm).

Per-core external layouts (host-prepped, all bf16 except gate/attn consts):
  xh   (8, 128, 2, 16, 128)  [cs2, hp, ht, c, w]   HC input, w-slice
  xw   (16, 128, 2, 8, 256)  [cs, wp, wt, c, h]    WC input, full w
  xres (16, 128, 2, 16, 128) [slab, cp, ot, hh, w] residual, w-slice
  out  (16, 128, 2, 16, 128) [slab, cp, ot, hh, w] bf16 output
  ysc  (2, 128, 2, 128, 128) [ct, cp, ht, hp, w]   DRAM scratch (bf16)
"""
import sys
import os

sys.path.insert(0, "/opt/trn_rl_repo")

import numpy as np
import ml_dtypes

import concourse.bacc as bacc
import concourse.mybir as mybir
import concourse.tile as tile
from concourse import bass_utils

F32 = mybir.dt.float32
BF16 = mybir.dt.bfloat16
AF = mybir.ActivationFunctionType
ALU = mybir.AluOpType

N = 256          # H = W
C2 = 128         # channels per branch
B = 4
NCORES = 8
WS = 128         # per-core w-slice width

_BF16_NP = ml_dtypes.bfloat16


def _dft_basis():
    """Orthonormal real DFT basis T (N, N): y = T^T diag(ghat) T x == irfft(G*rfft(x))."""
    n = np.arange(N)
    k = np.arange(1, N // 2)
    T = np.zeros((N, N), np.float64)
    T[0, :] = 1.0 / np.sqrt(N)
    T[1:N // 2, :] = np.sqrt(2.0 / N) * np.cos(2 * np.pi * k[:, None] * n[None, :] / N)
    T[N // 2, :] = (1.0 / np.sqrt(N)) * ((-1.0) ** n)
    T[N // 2 + 1:, :] = np.sqrt(2.0 / N) * np.sin(2 * np.pi * k[:, None] * n[None, :] / N)
    return T.astype(np.float32)


def _part_major(a):
    """(256, ...) -> (128, 2, ...) partition-major layout."""
    a = np.asarray(a)
    return np.ascontiguousarray(a.reshape(2, 128, *a.shape[1:]).transpose(
        (1, 0) + tuple(range(2, a.ndim + 1))))


_MLPS = ("ah", "bc1", "aw", "bc2")


def _build():
    nc = bacc.Bacc("TRN2", target_bir_lowering=False, num_devices=NCORES)

    # ---------------- I/O declarations ----------------
    xh_d = nc.dram_tensor("xh", [8, 128, 2, 16, 128], BF16, kind="ExternalInput")
    xw_d = nc.dram_tensor("xw", [16, 128, 2, 8, 256], BF16, kind="ExternalInput")
    xres_d = nc.dram_tensor("xres", [16, 128, 2, 16, 128], BF16, kind="ExternalInput")
    tfwd_d = nc.dram_tensor("tfwd", [128, 2, 256], BF16, kind="ExternalInput")
    tinv_d = nc.dram_tensor("tinv", [128, 2, 256], BF16, kind="ExternalInput")
    tinvw_d = nc.dram_tensor("tinvw", [128, 2, 128], BF16, kind="ExternalInput")
    sigw_d = nc.dram_tensor("sigw", [128, 2], F32, kind="ExternalInput")
    omega_d = nc.dram_tensor("omega", [1, 129], F32, kind="ExternalInput")
    lam_d = nc.dram_tensor("lam", [1, 128], F32, kind="ExternalInput")
    mlp_d = {}
    for m in _MLPS:
        mlp_d[m] = dict(
            w1t=nc.dram_tensor(f"{m}_w1t", [1, 64], F32, kind="ExternalInput"),
            b1=nc.dram_tensor(f"{m}_b1v", [64, 1], F32, kind="ExternalInput"),
            w2t=nc.dram_tensor(f"{m}_w2t", [64, 64], F32, kind="ExternalInput"),
            b2=nc.dram_tensor(f"{m}_b2v", [64, 1], F32, kind="ExternalInput"),
            w3t=nc.dram_tensor(f"{m}_w3t", [64, 8], F32, kind="ExternalInput"),
            b3=nc.dram_tensor(f"{m}_b3v", [8, 1], F32, kind="ExternalInput"),
        )
    caw1t_d = nc.dram_tensor("caw1t", [128, 2, 256], F32, kind="ExternalInput")
    cab1_d = nc.dram_tensor("cab1", [128, 2], F32, kind="ExternalInput")
    dwc_d = nc.dram_tensor("dwc", [128, 2], F32, kind="ExternalInput")
    dwb_d = nc.dram_tensor("dwb", [128, 2], F32, kind="ExternalInput")
    lcwt_d = nc.dram_tensor("lcwt", [128, 2, 256], F32, kind="ExternalInput")
    bng_d = nc.dram_tensor("bng", [128, 2], F32, kind="ExternalInput")
    bnb_d = nc.dram_tensor("bnb", [128, 2], F32, kind="ExternalInput")
    bnm_d = nc.dram_tensor("bnm", [128, 2], F32, kind="ExternalInput")
    bnv_d = nc.dram_tensor("bnv", [128, 2], F32, kind="ExternalInput")

    out_d = nc.dram_tensor("out", [16, 128, 2, 16, 128], BF16, kind="ExternalOutput")

    ysc_d = nc.dram_tensor("ysc", [2, 128, 2, 128, 128], BF16)  # (ct, cp, ht, hp, w)
    arh_in = nc.dram_tensor("arh_in", [128, 1], F32)
    arh_out = nc.dram_tensor("arh_out", [128, 1], F32)
    arw_in = nc.dram_tensor("arw_in", [128, 1], F32)
    arw_out = nc.dram_tensor("arw_out", [128, 1], F32)

    with tile.TileContext(nc) as tc:
        with tc.tile_pool(name="consts", bufs=1) as consts, \
             tc.tile_pool(name="xin", bufs=3) as xin, \
             tc.tile_pool(name="xhp", bufs=8) as xhp, \
             tc.tile_pool(name="uch", bufs=3) as uch, \
             tc.tile_pool(name="stg", bufs=2) as stg, \
             tc.tile_pool(name="crhs", bufs=2) as crhs, \
             tc.tile_pool(name="outp", bufs=2) as outp, \
             tc.tile_pool(name="gsb", bufs=1) as gsb, \
             tc.tile_pool(name="ps", bufs=2, space="PSUM") as ps:

            # ------- first WC input loads (kick these off before anything) ----
            xw_t_pre = []
            for cs in range(2):
                xt = xin.tile([128, 2, 8, 256], BF16, tag="xw")
                nc.sync.dma_start(out=xt, in_=xw_d[cs])
                xw_t_pre.append(xt)

            # ---------------- const loads (scalar HWDGE queue) ----------------
            # tiny gate-MLP inputs first so gate compute starts immediately
            omega_t = consts.tile([1, 129], F32, tag="omega")
            nc.scalar.dma_start(out=omega_t, in_=omega_d[:])
            lam_t = consts.tile([1, 128], F32, tag="lam")
            nc.scalar.dma_start(out=lam_t, in_=lam_d[:])
            tfwd_t = consts.tile([128, 2, 256], BF16, tag="tfwd")
            nc.scalar.dma_start(out=tfwd_t, in_=tfwd_d[:])
            tinvw_t = consts.tile([128, 2, 128], BF16, tag="tinvw")
            nc.scalar.dma_start(out=tinvw_t, in_=tinvw_d[:])
            # full HC input residency (8 MiB): streams on the scalar queue
            # during the WC phase so the HC phase never waits on loads
            xh_ts = []
            for cs2 in range(8):
                xt = xhp.tile([128, 2, 16, 128], BF16, tag="xh")
                nc.scalar.dma_start(out=xt, in_=xh_d[cs2])
                xh_ts.append(xt)
            tinv_t = consts.tile([128, 2, 256], BF16, tag="tinv")
            nc.scalar.dma_start(out=tinv_t, in_=tinv_d[:])
            sigw_t = consts.tile([128, 2], F32, tag="sigw")
            nc.scalar.dma_start(out=sigw_t, in_=sigw_d[:])
            caw1t_t = consts.tile([128, 2, 256], F32, tag="caw1t")
            nc.scalar.dma_start(out=caw1t_t, in_=caw1t_d[:])
            lcwt_t = consts.tile([128, 2, 256], F32, tag="lcwt")
            nc.scalar.dma_start(out=lcwt_t, in_=lcwt_d[:])
            vec_t = {}
            for nm, d in (("cab1", cab1_d), ("dwc", dwc_d), ("dwb", dwb_d),
                          ("bng", bng_d), ("bnb", bnb_d), ("bnm", bnm_d), ("bnv", bnv_d)):
                vt = consts.tile([128, 2], F32, tag=f"v_{nm}")
                nc.scalar.dma_start(out=vt, in_=d[:])
                vec_t[nm] = vt
            ones_t = consts.tile([128, 1], F32, tag="ones")
            nc.vector.memset(ones_t, 1.0)
            one1_t = consts.tile([1, 1], F32, tag="one1")
            nc.vector.memset(one1_t, 1.0)

            # ---------------- gate MLPs (tiny), table-batched ----------------
            def mlp_head(m, xvec, nk, role):
                d = mlp_d[m]
                w1t = gsb.tile([1, 64], F32, tag="m_w1")
                nc.scalar.dma_start(out=w1t, in_=d["w1t"][:])
                b1 = gsb.tile([64, 1], F32, tag="m_b1")
                nc.scalar.dma_start(out=b1, in_=d["b1"][:])
                w2t = gsb.tile([64, 64], F32, tag="m_w2")
                nc.scalar.dma_start(out=w2t, in_=d["w2t"][:])
                b2 = gsb.tile([64, 1], F32, tag="m_b2")
                nc.scalar.dma_start(out=b2, in_=d["b2"][:])
                w3t = gsb.tile([64, 8], F32, tag="m_w3")
                nc.scalar.dma_start(out=w3t, in_=d["w3t"][:])
                b3 = gsb.tile([8, 1], F32, tag="m_b3")
                nc.scalar.dma_start(out=b3, in_=d["b3"][:])

                p1 = ps.tile([64, nk], F32, tag="P0")
                nc.tensor.matmul(p1, lhsT=w1t, rhs=xvec, start=True, stop=True)
                h1 = gsb.tile([64, nk], F32, tag="m_h1")
                nc.scalar.activation(h1, p1, AF.Gelu, bias=b1)
                p2 = ps.tile([64, nk], F32, tag="P1")
                nc.tensor.matmul(p2, lhsT=w2t, rhs=h1, start=True, stop=True)
                h2 = gsb.tile([64, nk], F32, tag="m_h2")
                nc.scalar.activation(h2, p2, AF.Gelu, bias=b2)
                p3 = ps.tile([8, nk], F32, tag="P0")
                nc.tensor.matmul(p3, lhsT=w3t, rhs=h2, start=True, stop=True)
                at = gsb.tile([8, nk], F32, tag=f"m_at{role}")
                nc.scalar.activation(at, p3, AF.Identity, bias=b3)
                return at

            ghh = consts.tile([128, 2, 128], F32, tag="ghh")
            ghw = consts.tile([128, 2, 128], F32, tag="ghw")
            gp = {}
            gtags = {("h", 0): "P0", ("h", 1): "P1", ("w", 0): "P2", ("w", 1): "P3"}
            for (am, bm, nmk) in (("aw", "bc2", "w"), ("ah", "bc1", "h")):
                at = mlp_head(am, omega_t, 129, "a")
                bt = mlp_head(bm, lam_t, 128, "b")
                g0 = ps.tile([128, 128], F32, tag=gtags[(nmk, 0)])
                nc.tensor.matmul(g0, lhsT=at[:, 0:128], rhs=bt, start=True, stop=True)
                gn = ps.tile([1, 128], F32, tag=gtags[(nmk, 1)])
                nc.tensor.matmul(gn, lhsT=at[:, 128:129], rhs=bt, start=True, stop=True)
                gp[(nmk, 0)] = g0
                gp[(nmk, 1)] = gn
            # softplus(z) = relu(z) + log1p(exp(-|z|)), stage-batched across all 4
            keys = list(gp.keys())
            sp = {}
            for i, key in enumerate(keys):
                npart = 128 if key[1] == 0 else 1
                na = gsb.tile([128, 128], F32, tag=f"sp_na{i}")
                nc.scalar.activation(na[:npart, :], gp[key], AF.Abs)
                sp[key] = na
            for i, key in enumerate(keys):
                npart = 128 if key[1] == 0 else 1
                ex = gsb.tile([128, 128], F32, tag=f"sp_ex{i}")
                nc.scalar.activation(ex[:npart, :], sp[key][:npart, :], AF.Exp, scale=-1.0)
                nc.vector.tensor_scalar_add(ex[:npart, :], ex[:npart, :], 1.0)
                sp[key] = ex
            for key in keys:
                npart = 128 if key[1] == 0 else 1
                nc.scalar.activation(sp[key][:npart, :], sp[key][:npart, :], AF.Ln)
            for i, key in enumerate(keys):
                npart = 128 if key[1] == 0 else 1
                re = gsb.tile([128, 128], F32, tag=f"sp_re{i}")
                nc.scalar.activation(re[:npart, :], gp[key], AF.Relu)
                gh = ghh if key[0] == "h" else ghw
                if key[1] == 0:
                    nc.vector.tensor_add(gh[:, 0, :], sp[key][:128, :], re[:128, :])
                else:
                    # rows 128+j of ghat equal G[j]: copy the aligned block first,
                    # then overwrite row 0 with the Nyquist G[128].
                    nc.vector.tensor_copy(gh[:, 1, :], gh[:, 0, :])
                    nc.vector.tensor_add(gh[0:1, 1, :], sp[key][0:1, :], re[0:1, :])
            for gh in (ghh, ghw):
                nc.vector.tensor_scalar_mul(gh[:, :, :], gh[:, :, :], float(8.0 ** -0.5))

            # ---------------- BN prep ----------------
            bninv = consts.tile([128, 2], F32, tag="bninv")
            nc.vector.tensor_scalar_add(bninv, vec_t["bnv"], 1e-5)
            nc.scalar.activation(bninv, bninv, AF.Sqrt)
            nc.vector.reciprocal(bninv, bninv)
            nc.vector.tensor_tensor(out=bninv, in0=vec_t["bng"], in1=bninv, op=ALU.mult)
            bnbeff = consts.tile([128, 2], F32, tag="bnbeff")
            nc.vector.tensor_tensor(out=bnbeff, in0=vec_t["bnm"], in1=bninv, op=ALU.mult)
            nc.vector.tensor_tensor(out=bnbeff, in0=vec_t["bnb"], in1=bnbeff, op=ALU.subtract)

            xsum_h = consts.tile([128, 2, 128], F32, tag="xsumh")      # [h, ht, c]
            xsum_w = consts.tile([128, 2, 128], F32, tag="xsumw")      # [w, wt, c]

            # ---------------- WC branch (first: its pooled sums gate the AR) ----
            for cs in range(16):
                if cs < 2:
                    xw_t = xw_t_pre[cs]
                else:
                    xw_t = xin.tile([128, 2, 8, 256], BF16, tag="xw")
                    nc.sync.dma_start(out=xw_t, in_=xw_d[cs])
                # h-sum via GPSIMD tree-adds (keeps DVE free)
                for wt in (0, 1):
                    tr1 = gsb.tile([128, 8, 128], BF16, tag=f"tree1{wt}")
                    nc.gpsimd.tensor_add(tr1, xw_t[:, wt, :, 0:128], xw_t[:, wt, :, 128:256])
                    tr2 = gsb.tile([128, 8, 64], BF16, tag=f"tree2{wt}")
                    nc.gpsimd.tensor_add(tr2, tr1[:, :, 0:64], tr1[:, :, 64:128])
                    tr3 = gsb.tile([128, 8, 32], BF16, tag=f"tree3{wt}")
                    nc.gpsimd.tensor_add(tr3, tr2[:, :, 0:32], tr2[:, :, 32:64])
                    nc.vector.tensor_reduce(out=xsum_w[:, wt, cs * 8:cs * 8 + 8], in_=tr3,
                                            axis=mybir.AxisListType.X, op=ALU.add)
                ystg = []
                for ht in (0, 1):
                    st = stg.tile([128, 8, 128], BF16, tag=f"ystgw{ht}")
                    ystg.append(st)
                for ccp in (0, 4):
                    ugw = {}
                    for cc in (ccp, ccp + 2):
                        c0 = cs * 8 + cc
                        # forward transform: u[k, c, h], accumulate over wt
                        for kt in (0, 1):
                            pk = ps.tile([128, 2, 256], F32, tag=f"P{kt}")
                            for wt in (0, 1):
                                nc.tensor.matmul(pk,
                                                 lhsT=tfwd_t[:, wt, kt * 128:(kt + 1) * 128],
                                                 rhs=xw_t[:, wt, cc:cc + 2, :],
                                                 start=(wt == 0), stop=(wt == 1))
                            # gate on DVE (psum -> bf16 u)
                            u = uch.tile([128, 2, 256], BF16, tag=f"uw{kt}")
                            nc.vector.tensor_tensor(
                                out=u, in0=pk,
                                in1=ghw[:, kt, c0:c0 + 2].unsqueeze(2)
                                .broadcast_to([128, 2, 256]),
                                op=ALU.mult)
                            ugw[(cc, kt)] = u
                    # inverse transform restricted to our w-slice, h-major out;
                    # one full PSUM bank (4 channels) per ht, drained by ACT
                    for ht in (0, 1):
                        pyw = ps.tile([128, 4, 128], F32, tag=f"P{2 + ht}")
                        for ci in range(4):
                            cc, c2 = ccp + 2 * (ci // 2), ci % 2
                            for kt in (0, 1):
                                nc.tensor.matmul(pyw[:, ci, :],
                                                 lhsT=ugw[(cc, kt)][:, c2,
                                                                    ht * 128:(ht + 1) * 128],
                                                 rhs=tinvw_t[:, kt, :],
                                                 start=(kt == 0), stop=(kt == 1))
                        nc.scalar.activation(ystg[ht][:, ccp:ccp + 4, :], pyw, AF.Copy)
                for ht in (0, 1):
                    nc.sync.dma_start(
                        out=ysc_d[1, cs * 8:cs * 8 + 8, ht, :, :]
                        .rearrange("c h w -> h c w"),
                        in_=ystg[ht])

            # pool_w = sum_k sigw[k] ghw[k, c] * (T @ xsum_w)[k, c]  -> AllReduce #1
            xsum_wb = gsb.tile([128, 2, 128], BF16, tag="xsumwb")
            nc.vector.tensor_copy(xsum_wb, xsum_w)
            t1 = []
            for kt in (0, 1):
                m1 = ps.tile([128, 128], F32, tag=f"P{2 + kt}")
                for wt in (0, 1):
                    nc.tensor.matmul(m1, lhsT=tfwd_t[:, wt, kt * 128:(kt + 1) * 128],
                                     rhs=xsum_wb[:, wt, :], start=(wt == 0), stop=(wt == 1))
                tt = gsb.tile([128, 128], F32, tag=f"t1_{kt}")
                nc.vector.tensor_tensor(out=tt, in0=m1, in1=ghw[:, kt, :], op=ALU.mult)
                t1.append(tt)
            pw_ps = ps.tile([128, 1], F32, tag="P0")
            for kt in (0, 1):
                nc.tensor.matmul(pw_ps, lhsT=t1[kt], rhs=sigw_t[:, kt:kt + 1],
                                 start=(kt == 0), stop=(kt == 1))
            poolw_sb = gsb.tile([128, 1], F32, tag="poolw")
            nc.vector.tensor_copy(poolw_sb, pw_ps)
            nc.sync.dma_start(out=arw_in[:], in_=poolw_sb)
            nc.gpsimd.collective_compute(
                "AllReduce", ALU.add,
                replica_groups=[[0, 1], [2, 3], [4, 5], [6, 7]],
                ins=[arw_in[:]], outs=[arw_out[:]])

            # xsum_h from the resident xh tiles (loads complete during WC),
            # so AllReduce #2 + attention + wsc overlap the HC compute phase
            for cs2 in range(8):
                for ht in (0, 1):
                    nc.vector.tensor_reduce(out=xsum_h[:, ht, cs2 * 16:cs2 * 16 + 16],
                                            in_=xh_ts[cs2][:, ht, :, :],
                                            axis=mybir.AxisListType.X, op=ALU.add)
            # pool_h = ghh[0, :] * sum_{h,w} xh  -> AllReduce #2
            ph_ps = ps.tile([128, 1], F32, tag="P0")
            for ht in (0, 1):
                nc.tensor.matmul(ph_ps, lhsT=xsum_h[:, ht, :], rhs=ones_t,
                                 start=(ht == 0), stop=(ht == 1))
            g0_ps = ps.tile([128, 1], F32, tag="P1")
            nc.tensor.matmul(g0_ps, lhsT=ghh[0:1, 0, :], rhs=one1_t, start=True, stop=True)
            g0_sb = gsb.tile([128, 1], F32, tag="g0sb")
            nc.vector.tensor_copy(g0_sb, g0_ps)
            poolh_sb = gsb.tile([128, 1], F32, tag="poolh")
            nc.vector.tensor_tensor(out=poolh_sb, in0=ph_ps, in1=g0_sb, op=ALU.mult)
            nc.sync.dma_start(out=arh_in[:], in_=poolh_sb)
            nc.gpsimd.collective_compute(
                "AllReduce", ALU.add,
                replica_groups=[[0, 1], [2, 3], [4, 5], [6, 7]],
                ins=[arh_in[:]], outs=[arh_out[:]])

            p_sb = []
            for ct, aro in ((0, arh_out), (1, arw_out)):
                pt = gsb.tile([128, 1], F32, tag=f"p_ar{ct}")
                nc.sync.dma_start(out=pt, in_=aro[:])
                p_sb.append(pt)

            # ---------------- channel attention -> folded conv weights ----------------
            q_sb = []
            for ot in (0, 1):
                q_ps = ps.tile([128, 1], F32, tag=f"P{2 + ot}")
                for ct in (0, 1):
                    nc.tensor.matmul(q_ps, lhsT=caw1t_t[:, ct, ot * 128:(ot + 1) * 128],
                                     rhs=p_sb[ct], start=(ct == 0), stop=(ct == 1))
                qt = gsb.tile([128, 1], F32, tag=f"q{ot}")
                nc.scalar.activation(qt, q_ps, AF.Gelu, bias=vec_t["cab1"][:, ot:ot + 1])
                nc.vector.tensor_tensor(out=qt, in0=qt, in1=vec_t["dwc"][:, ot:ot + 1],
                                        op=ALU.mult)
                q_sb.append(qt)
            s_sb = []
            for ot in (0, 1):
                s_t = gsb.tile([128, 1], F32, tag=f"s{ot}")
                nc.scalar.activation(s_t, q_sb[ot], AF.Sigmoid, bias=vec_t["dwb"][:, ot:ot + 1])
                s_sb.append(s_t)
            wsc = consts.tile([128, 2, 256], BF16, tag="wsc")
            for ct in (0, 1):
                nc.vector.tensor_scalar_mul(wsc[:, ct, :], lcwt_t[:, ct, :], s_sb[ct])

            # ---------------- HC branch ----------------
            for cs2 in range(8):
                xh_t = xh_ts[cs2]
                ystg = []
                for ht in (0, 1):
                    st = stg.tile([128, 16, 128], BF16, tag=f"ystgh{ht}")
                    ystg.append(st)
                for cc in range(0, 16, 4):
                    gc = cs2 * 16 + cc
                    ug = []
                    for kt in (0, 1):
                        pk = ps.tile([128, 4, 128], F32, tag=f"P{kt}")
                        for ht in (0, 1):
                            nc.tensor.matmul(pk, lhsT=tfwd_t[:, ht, kt * 128:(kt + 1) * 128],
                                             rhs=xh_t[:, ht, cc:cc + 4, :],
                                             start=(ht == 0), stop=(ht == 1))
                        u = uch.tile([128, 4, 128], BF16, tag=f"ug{kt}")
                        nc.vector.tensor_tensor(
                            out=u, in0=pk,
                            in1=ghh[:, kt, gc:gc + 4].unsqueeze(2)
                            .broadcast_to([128, 4, 128]),
                            op=ALU.mult)
                        ug.append(u)
                    for ht in (0, 1):
                        py = ps.tile([128, 4, 128], F32, tag=f"P{2 + ht}")
                        for kt in (0, 1):
                            nc.tensor.matmul(py, lhsT=tinv_t[:, kt, ht * 128:(ht + 1) * 128],
                                             rhs=ug[kt], start=(kt == 0), stop=(kt == 1))
                        nc.scalar.activation(ystg[ht][:, cc:cc + 4, :], py, AF.Copy)
                for ht in (0, 1):
                    nc.sync.dma_start(
                        out=ysc_d[0, cs2 * 16:cs2 * 16 + 16, ht, :, :]
                        .rearrange("c h w -> h c w"),
                        in_=ystg[ht])

            # ---------------- conv 1x1 + BN + GELU + residual add ----------------
            for slab in range(16):
                ht, hs = slab // 8, (slab % 8) * 16
                rts = []
                for ct in (0, 1):
                    rt = crhs.tile([128, 16, 128], BF16, tag=f"cr{ct}")
                    nc.scalar.dma_start(out=rt, in_=ysc_d[ct, :, ht, hs:hs + 16, :])
                    rts.append(rt)
                xt = outp.tile([128, 2, 16, 128], BF16, tag="xres")
                nc.sync.dma_start(out=xt, in_=xres_d[slab])
                for ot in (0, 1):
                    gstg = outp.tile([128, 16, 128], BF16, tag=f"gstg{ot}")
                    for sl in range(0, 16, 4):
                        po = ps.tile([128, 4, 128], F32, tag=f"P{2 * ot + (sl // 4) % 2}")
                        for ct in (0, 1):
                            nc.tensor.matmul(po, lhsT=wsc[:, ct, ot * 128:(ot + 1) * 128],
                                             rhs=rts[ct][:, sl:sl + 4, :],
                                             start=(ct == 0), stop=(ct == 1))
                        nc.scalar.activation(gstg[:, sl:sl + 4, :], po, AF.Gelu,
                                             bias=bnbeff[:, ot:ot + 1],
                                             scale=bninv[:, ot:ot + 1])
                    # residual add (one wide DVE op), then store
                    nc.vector.tensor_add(gstg, gstg, xt[:, ot, :, :])
                    nc.sync.dma_start(out=out_d[slab, :, ot, :, :], in_=gstg)

    nc.compile()
    return nc


_NC_CACHE = None


def _get_nc():
    global _NC_CACHE
    if _NC_CACHE is None:
        _NC_CACHE = _build()
    return _NC_CACHE


def _host_consts(inputs, core):
    """Per-core constant inputs (everything except the x shards)."""
    s = core % 2
    wlo = WS * s
    T = _dft_basis()
    d = {}
    d["tfwd"] = _part_major(np.ascontiguousarray(T.T)).astype(_BF16_NP)
    d["tinv"] = _part_major(T).astype(_BF16_NP)
    d["tinvw"] = _part_major(np.ascontiguousarray(T[:, wlo:wlo + WS])).astype(_BF16_NP)
    d["sigw"] = _part_major(T[:, wlo:wlo + WS].sum(axis=1)).astype(np.float32)
    d["omega"] = (np.arange(129, dtype=np.float32) / 128.0 - 1.0).reshape(1, 129)
    d["lam"] = np.linspace(-1.0, 1.0, 128, dtype=np.float32).reshape(1, 128)
    for m in _MLPS:
        d[f"{m}_w1t"] = np.ascontiguousarray(inputs[f"{m}_w1"].T).astype(np.float32)
        d[f"{m}_b1v"] = inputs[f"{m}_b1"].reshape(64, 1).astype(np.float32)
        d[f"{m}_w2t"] = np.ascontiguousarray(inputs[f"{m}_w2"].T).astype(np.float32)
        d[f"{m}_b2v"] = inputs[f"{m}_b2"].reshape(64, 1).astype(np.float32)
        d[f"{m}_w3t"] = np.ascontiguousarray(inputs[f"{m}_w3"].T).astype(np.float32)
        d[f"{m}_b3v"] = inputs[f"{m}_b3"].reshape(8, 1).astype(np.float32)
    d["caw1t"] = _part_major(np.ascontiguousarray(inputs["ca_w1"].T) / 65536.0).astype(np.float32)
    d["cab1"] = _part_major(inputs["ca_b1"]).astype(np.float32)
    d["dwc"] = _part_major(np.ascontiguousarray(inputs["ca_dw"][:, 1, 1])).astype(np.float32)
    d["dwb"] = _part_major(inputs["ca_db"]).astype(np.float32)
    d["lcwt"] = _part_major(np.ascontiguousarray(inputs["lc_w"].T)).astype(np.float32)
    d["bng"] = _part_major(inputs["bn_g"]).astype(np.float32)
    d["bnb"] = _part_major(inputs["bn_b"]).astype(np.float32)
    d["bnm"] = _part_major(inputs["bn_m"]).astype(np.float32)
    d["bnv"] = _part_major(inputs["bn_v"]).astype(np.float32)
    return d


def kernel(**inputs):
    x = np.asarray(inputs["x"], np.float32)
    nc = _get_nc()

    in_maps = []
    for core in range(NCORES):
        b, s = core // 2, core % 2
        wlo = WS * s
        m = _host_consts(inputs, core)
        xb16 = x[b].astype(_BF16_NP)
        # xh: (8cs2, 128hp, 2ht, 16c, 128w) from x[b, :128, :, wsl]
        m["xh"] = np.ascontiguousarray(
            xb16[:C2, :, wlo:wlo + WS]
            .reshape(8, 16, 2, 128, 128).transpose(0, 3, 2, 1, 4))
        # xw: (16cs, 128wp, 2wt, 8c, 256h) from x[b, 128:, :, :]
        m["xw"] = np.ascontiguousarray(
            xb16[C2:, :, :]
            .reshape(16, 8, 256, 2, 128).transpose(0, 4, 3, 1, 2))
        # xres: (16slab, 128cp, 2ot, 16hh, 128w) from x[b, :, :, wsl]
        m["xres"] = np.ascontiguousarray(
            xb16[:, :, wlo:wlo + WS]
            .reshape(2, 128, 16, 16, 128).transpose(2, 1, 0, 3, 4))
        in_maps.append(m)

    trace = os.environ.get("BASS_KERNEL_TRACE", "0") == "1"
    res = bass_utils.run_bass_kernel_spmd(
        nc, in_maps, core_ids=list(range(NCORES)),
        trace=trace, trace_cores=list(range(NCORES)) if trace else None,
        stitch_traces=False)
    if trace and res.exec_time_ns is not None:
        print(f"HW exec time: {res.exec_time_ns} ns")
        print(f"   mean exec time: {res.mean_exec_time_ns} ns  "
              f"(slowest core {res.max_exec_time_core_id})")
        if res.instructions_and_trace is not None:
            print("   trace:", res.instructions_and_trace[1])

    out = np.empty((B, 2 * C2, N, N), np.float32)
    for core in range(NCORES):
        b, s = core // 2, core % 2
        wlo = WS * s
        o = np.asarray(res.results[core]["out"])  # (16, 128, 2, 16, 128) bf16
        out[b, :, :, wlo:wlo + WS] = (
            o.transpose(2, 1, 0, 3, 4).reshape(256, 256, 128).astype(np.float32))
    return out


# revision 19
# speedup vs baseline: 1.1526x; 1.1526x over previous
"""Trainium2 Bass kernel for the spectral-gating network (nn_DAPSO).

Model (B=4, C=256, H=W=256):
  - channels 0:128   : y_h = irfft(Gh * rfft(x, axis=H))   (per-channel gate)
  - channels 128:256 : y_w = irfft(Gw * rfft(x, axis=W))
  - gates Gh/Gw from tiny MLPs (computed on device)
  - channel attention: s = sigmoid(dw(gelu(W1 @ mean_hw(y) + b)))  -> y *= s
  - y2 = gelu(BN(lc_w @ y));  out = x + y2

Key algorithmic mapping: irfft(G*rfft(x)) along an axis of length N equals
T^T diag(ghat) T x with T the orthonormal real DFT basis (cos/sin rows), so
both branches become dense TensorE matmuls (no FFT).

Sharding: 8 cores = 4 batches x 2 w-halves. Each core computes BOTH branch
outputs for its (batch, w-half) spatial region: the H-branch needs only its
w-columns; the W-branch contracts the full W axis (its forward transform is
duplicated between the pair of cores). The only cross-core communication is
a 1KB AllReduce of the pooled channel means.

Channel attention pooling is computed analytically from input sums (DC
coefficient trick), so it never blocks on branch outputs.

v2 layout/throughput rework vs the original baseline (799us):
  - no f32 residual pre-copy / accumulate-DMA: the conv stage loads x as
    bf16 tiles, adds the residual on GpSimd, and writes a bf16 output
    (host upcasts). Saves ~96 MiB/core of DRAM traffic.
  - all external tensors host-pre-tiled so every DMA is contiguous per
    partition; the branch->conv scratch is c-major so the DMA-transpose
    cost sits on the (PE-bound) branch phase, not the (DMA-bound) conv
    phase.
  - matmul loops restructured for dense PE streams (HAM stays warm).

Per-core external layouts (host-prepped, all bf16 except gate/attn consts):
  xh   (8, 128, 2, 16, 128)  [cs2, hp, ht, c, w]   HC input, w-slice
  xw   (16, 128, 2, 8, 256)  [cs, wp, wt, c, h]    WC input, full w
  xres (16, 128, 2, 16, 128) [slab, cp, ot, hh, w] residual, w-slice
  out  (16, 128, 2, 16, 128) [slab, cp, ot, hh, w] bf16 output
  ysc  (2, 128, 2, 128, 128) [ct, cp, ht, hp, w]   DRAM scratch (bf16)
"""
import sys
import os

sys.path.insert(0, "/opt/trn_rl_repo")

import numpy as np
import ml_dtypes

import concourse.bacc as bacc
import concourse.mybir as mybir
import concourse.tile as tile
from concourse import bass_utils

F32 = mybir.dt.float32
BF16 = mybir.dt.bfloat16
AF = mybir.ActivationFunctionType
ALU = mybir.AluOpType

N = 256          # H = W
C2 = 128         # channels per branch
B = 4
NCORES = 8
WS = 128         # per-core w-slice width

_BF16_NP = ml_dtypes.bfloat16


def _dft_basis():
    """Orthonormal real DFT basis T (N, N): y = T^T diag(ghat) T x == irfft(G*rfft(x))."""
    n = np.arange(N)
    k = np.arange(1, N // 2)
    T = np.zeros((N, N), np.float64)
    T[0, :] = 1.0 / np.sqrt(N)
    T[1:N // 2, :] = np.sqrt(2.0 / N) * np.cos(2 * np.pi * k[:, None] * n[None, :] / N)
    T[N // 2, :] = (1.0 / np.sqrt(N)) * ((-1.0) ** n)
    T[N // 2 + 1:, :] = np.sqrt(2.0 / N) * np.sin(2 * np.pi * k[:, None] * n[None, :] / N)
    return T.astype(np.float32)


def _part_major(a):
    """(256, ...) -> (128, 2, ...) partition-major layout."""
    a = np.asarray(a)
    return np.ascontiguousarray(a.reshape(2, 128, *a.shape[1:]).transpose(
        (1, 0) + tuple(range(2, a.ndim + 1))))


_MLPS = ("ah", "bc1", "aw", "bc2")


def _build():
    nc = bacc.Bacc("TRN2", target_bir_lowering=False, num_devices=NCORES)

    # ---------------- I/O declarations ----------------
    xh_d = nc.dram_tensor("xh", [8, 128, 2, 16, 128], BF16, kind="ExternalInput")
    xw_d = nc.dram_tensor("xw", [16, 128, 2, 8, 256], BF16, kind="ExternalInput")
    xres_d = nc.dram_tensor("xres", [16, 128, 2, 16, 128], BF16, kind="ExternalInput")
    tfwd_d = nc.dram_tensor("tfwd", [128, 2, 256], BF16, kind="ExternalInput")
    tinv_d = nc.dram_tensor("tinv", [128, 2, 256], BF16, kind="ExternalInput")
    tinvw_d = nc.dram_tensor("tinvw", [128, 2, 128], BF16, kind="ExternalInput")
    sigw_d = nc.dram_tensor("sigw", [128, 2], F32, kind="ExternalInput")
    omega_d = nc.dram_tensor("omega", [1, 129], F32, kind="ExternalInput")
    lam_d = nc.dram_tensor("lam", [1, 128], F32, kind="ExternalInput")
    mlp_d = {}
    for m in _MLPS:
        mlp_d[m] = dict(
            w1t=nc.dram_tensor(f"{m}_w1t", [1, 64], F32, kind="ExternalInput"),
            b1=nc.dram_tensor(f"{m}_b1v", [64, 1], F32, kind="ExternalInput"),
            w2t=nc.dram_tensor(f"{m}_w2t", [64, 64], F32, kind="ExternalInput"),
            b2=nc.dram_tensor(f"{m}_b2v", [64, 1], F32, kind="ExternalInput"),
            w3t=nc.dram_tensor(f"{m}_w3t", [64, 8], F32, kind="ExternalInput"),
            b3=nc.dram_tensor(f"{m}_b3v", [8, 1], F32, kind="ExternalInput"),
        )
    caw1t_d = nc.dram_tensor("caw1t", [128, 2, 256], F32, kind="ExternalInput")
    cab1_d = nc.dram_tensor("cab1", [128, 2], F32, kind="ExternalInput")
    dwc_d = nc.dram_tensor("dwc", [128, 2], F32, kind="ExternalInput")
    dwb_d = nc.dram_tensor("dwb", [128, 2], F32, kind="ExternalInput")
    lcwt_d = nc.dram_tensor("lcwt", [128, 2, 256], F32, kind="ExternalInput")
    bng_d = nc.dram_tensor("bng", [128, 2], F32, kind="ExternalInput")
    bnb_d = nc.dram_tensor("bnb", [128, 2], F32, kind="ExternalInput")
    bnm_d = nc.dram_tensor("bnm", [128, 2], F32, kind="ExternalInput")
    bnv_d = nc.dram_tensor("bnv", [128, 2], F32, kind="ExternalInput")

    out_d = nc.dram_tensor("out", [16, 128, 2, 16, 128], BF16, kind="ExternalOutput")

    ysc_d = nc.dram_tensor("ysc", [2, 128, 2, 128, 128], BF16)  # (ct, cp, ht, hp, w)
    arh_in = nc.dram_tensor("arh_in", [128, 1], F32)
    arh_out = nc.dram_tensor("arh_out", [128, 1], F32)
    arw_in = nc.dram_tensor("arw_in", [128, 1], F32)
    arw_out = nc.dram_tensor("arw_out", [128, 1], F32)

    with tile.TileContext(nc) as tc:
        with tc.tile_pool(name="consts", bufs=1) as consts, \
             tc.tile_pool(name="xin", bufs=4) as xin, \
             tc.tile_pool(name="xhp", bufs=8) as xhp, \
             tc.tile_pool(name="uch", bufs=3) as uch, \
             tc.tile_pool(name="stg", bufs=2) as stg, \
             tc.tile_pool(name="crhs", bufs=2) as crhs, \
             tc.tile_pool(name="outp", bufs=2) as outp, \
             tc.tile_pool(name="gsb", bufs=1) as gsb, \
             tc.tile_pool(name="ps", bufs=2, space="PSUM") as ps:

            def pst(shape, tag, name):
                return ps.tile(shape, F32, tag=tag, bufs=3 if tag == "PF" else 2,
                               name=name)

            # ------- first WC input loads (kick these off before anything) ----
            xw_t_pre = []
            for cs in range(2):
                xt = xin.tile([128, 2, 8, 256], BF16, tag="xw")
                nc.sync.dma_start(out=xt, in_=xw_d[cs])
                xw_t_pre.append(xt)

            # ---------------- const loads (scalar HWDGE queue) ----------------
            # tiny gate-MLP inputs first so gate compute starts immediately
            omega_t = consts.tile([1, 129], F32, tag="omega")
            nc.scalar.dma_start(out=omega_t, in_=omega_d[:])
            lam_t = consts.tile([1, 128], F32, tag="lam")
            nc.scalar.dma_start(out=lam_t, in_=lam_d[:])
            mlp_t = {}
            for m in _MLPS:
                d = mlp_d[m]
                for nm, shp in (("w1t", [1, 64]), ("b1", [64, 1]), ("w2t", [64, 64]),
                                ("b2", [64, 1]), ("w3t", [64, 8]), ("b3", [8, 1])):
                    t = gsb.tile(shp, F32, tag=f"m_{m}_{nm}")
                    nc.scalar.dma_start(out=t, in_=d[nm][:])
                    mlp_t[(m, nm)] = t
            tfwd_t = consts.tile([128, 2, 256], BF16, tag="tfwd")
            nc.scalar.dma_start(out=tfwd_t, in_=tfwd_d[:])
            tinvw_t = consts.tile([128, 2, 128], BF16, tag="tinvw")
            nc.scalar.dma_start(out=tinvw_t, in_=tinvw_d[:])
            tinv_t = consts.tile([128, 2, 256], BF16, tag="tinv")
            nc.scalar.dma_start(out=tinv_t, in_=tinv_d[:])
            sigw_t = consts.tile([128, 2], F32, tag="sigw")
            nc.scalar.dma_start(out=sigw_t, in_=sigw_d[:])
            caw1t_t = consts.tile([128, 2, 256], F32, tag="caw1t")
            nc.scalar.dma_start(out=caw1t_t, in_=caw1t_d[:])
            lcwt_t = consts.tile([128, 2, 256], F32, tag="lcwt")
            nc.scalar.dma_start(out=lcwt_t, in_=lcwt_d[:])
            vec_t = {}
            for nm, d in (("cab1", cab1_d), ("dwc", dwc_d), ("dwb", dwb_d),
                          ("bng", bng_d), ("bnb", bnb_d), ("bnm", bnm_d), ("bnv", bnv_d)):
                vt = consts.tile([128, 2], F32, tag=f"v_{nm}")
                nc.scalar.dma_start(out=vt, in_=d[:])
                vec_t[nm] = vt
            ones_t = consts.tile([128, 1], F32, tag="ones")
            nc.vector.memset(ones_t, 1.0)
            one1_t = consts.tile([1, 1], F32, tag="one1")
            nc.vector.memset(one1_t, 1.0)
            # full HC input residency (8 MiB): streams on the scalar queue
            # during the WC phase so the HC phase never waits on loads
            xh_ts = []
            for cs2 in range(8):
                xt = xhp.tile([128, 2, 16, 128], BF16, tag="xh")
                nc.scalar.dma_start(out=xt, in_=xh_d[cs2])
                xh_ts.append(xt)

            # ---------------- gate MLPs (tiny), table-batched ----------------
            def mlp_head(m, xvec, nk, role):
                w1t, b1 = mlp_t[(m, "w1t")], mlp_t[(m, "b1")]
                w2t, b2 = mlp_t[(m, "w2t")], mlp_t[(m, "b2")]
                w3t, b3 = mlp_t[(m, "w3t")], mlp_t[(m, "b3")]

                p1 = pst([64, nk], "PF", "p1")
                nc.tensor.matmul(p1, lhsT=w1t, rhs=xvec, start=True, stop=True)
                h1 = gsb.tile([64, nk], F32, tag="m_h1")
                nc.scalar.activation(h1, p1, AF.Gelu, bias=b1)
                p2 = pst([64, nk], "PI", "p2")
                nc.tensor.matmul(p2, lhsT=w2t, rhs=h1, start=True, stop=True)
                h2 = gsb.tile([64, nk], F32, tag="m_h2")
                nc.scalar.activation(h2, p2, AF.Gelu, bias=b2)
                p3 = pst([8, nk], "PF", "p3")
                nc.tensor.matmul(p3, lhsT=w3t, rhs=h2, start=True, stop=True)
                at = gsb.tile([8, nk], F32, tag=f"m_at{role}")
                nc.scalar.activation(at, p3, AF.Identity, bias=b3)
                return at

            ghh = consts.tile([128, 2, 128], F32, tag="ghh")
            ghw = consts.tile([128, 2, 128], F32, tag="ghw")
            gp = {}
            gtags = {("h", 0): "PF", ("h", 1): "PI", ("w", 0): "PF", ("w", 1): "PI"}
            for (am, bm, nmk) in (("aw", "bc2", "w"), ("ah", "bc1", "h")):
                at = mlp_head(am, omega_t, 129, "a")
                bt = mlp_head(bm, lam_t, 128, "b")
                g0 = pst([128, 128], gtags[(nmk, 0)], "g0")
                nc.tensor.matmul(g0, lhsT=at[:, 0:128], rhs=bt, start=True, stop=True)
                gn = pst([1, 128], gtags[(nmk, 1)], "gn")
                nc.tensor.matmul(gn, lhsT=at[:, 128:129], rhs=bt, start=True, stop=True)
                gp[(nmk, 0)] = g0
                gp[(nmk, 1)] = gn
            # softplus(z) = relu(z) + log1p(exp(-|z|)), stage-batched across all 4
            keys = list(gp.keys())
            sp = {}
            for i, key in enumerate(keys):
                npart = 128 if key[1] == 0 else 1
                na = gsb.tile([128, 128], F32, tag=f"sp_na{i}")
                nc.scalar.activation(na[:npart, :], gp[key], AF.Abs)
                sp[key] = na
            for i, key in enumerate(keys):
                npart = 128 if key[1] == 0 else 1
                ex = gsb.tile([128, 128], F32, tag=f"sp_ex{i}")
                nc.scalar.activation(ex[:npart, :], sp[key][:npart, :], AF.Exp, scale=-1.0)
                nc.vector.tensor_scalar_add(ex[:npart, :], ex[:npart, :], 1.0)
                sp[key] = ex
            for key in keys:
                npart = 128 if key[1] == 0 else 1
                nc.scalar.activation(sp[key][:npart, :], sp[key][:npart, :], AF.Ln)
            for i, key in enumerate(keys):
                npart = 128 if key[1] == 0 else 1
                re = gsb.tile([128, 128], F32, tag=f"sp_re{i}")
                nc.scalar.activation(re[:npart, :], gp[key], AF.Relu)
                gh = ghh if key[0] == "h" else ghw
                if key[1] == 0:
                    nc.vector.tensor_add(gh[:, 0, :], sp[key][:128, :], re[:128, :])
                else:
                    # rows 128+j of ghat equal G[j]: copy the aligned block first,
                    # then overwrite row 0 with the Nyquist G[128].
                    nc.vector.tensor_copy(gh[:, 1, :], gh[:, 0, :])
                    nc.vector.tensor_add(gh[0:1, 1, :], sp[key][0:1, :], re[0:1, :])
            for gh in (ghh, ghw):
                nc.vector.tensor_scalar_mul(gh[:, :, :], gh[:, :, :], float(8.0 ** -0.5))

            # ---------------- BN prep ----------------
            bninv = consts.tile([128, 2], F32, tag="bninv")
            nc.vector.tensor_scalar_add(bninv, vec_t["bnv"], 1e-5)
            nc.scalar.activation(bninv, bninv, AF.Sqrt)
            nc.vector.reciprocal(bninv, bninv)
            nc.vector.tensor_tensor(out=bninv, in0=vec_t["bng"], in1=bninv, op=ALU.mult)
            bnbeff = consts.tile([128, 2], F32, tag="bnbeff")
            nc.vector.tensor_tensor(out=bnbeff, in0=vec_t["bnm"], in1=bninv, op=ALU.mult)
            nc.vector.tensor_tensor(out=bnbeff, in0=vec_t["bnb"], in1=bnbeff, op=ALU.subtract)

            xsum_h = consts.tile([128, 2, 128], F32, tag="xsumh")      # [h, ht, c]
            xsum_w = consts.tile([128, 2, 128], F32, tag="xsumw")      # [w, wt, c]

            # ---------------- WC branch (first: its pooled sums gate the AR) ----
            for cs in range(16):
                if cs < 2:
                    xw_t = xw_t_pre[cs]
                else:
                    xw_t = xin.tile([128, 2, 8, 256], BF16, tag="xw")
                    nc.sync.dma_start(out=xw_t, in_=xw_d[cs])
                # h-sum: one GPSIMD tree level (frees the xw slot fast), DVE reduce
                for wt in (0, 1):
                    tr1 = gsb.tile([128, 8, 128], BF16, tag=f"tree1{wt}")
                    nc.gpsimd.tensor_add(tr1, xw_t[:, wt, :, 0:128], xw_t[:, wt, :, 128:256])
                    nc.vector.tensor_reduce(out=xsum_w[:, wt, cs * 8:cs * 8 + 8], in_=tr1,
                                            axis=mybir.AxisListType.X, op=ALU.add)
                ystg2 = stg.tile([128, 2, 8, 128], BF16, tag="ystgw")
                for ccp in (0, 4):
                    # forward transform: u[k, c, h]; 2-bank psum per 2-channel
                    # group, weight-reuse order (each lhsT serves 2 matmuls)
                    pf = {}
                    for cc in (ccp, ccp + 2):
                        pf[cc] = pst([128, 2, 2, 256], "PF", "pfw")
                    for kt in (0, 1):
                        for wt in (0, 1):
                            for cc in (ccp, ccp + 2):
                                nc.tensor.matmul(
                                    pf[cc][:, kt],
                                    lhsT=tfwd_t[:, wt, kt * 128:(kt + 1) * 128],
                                    rhs=xw_t[:, wt, cc:cc + 2, :],
                                    start=(wt == 0), stop=(wt == 1))
                    ugw = {}
                    for cc in (ccp, ccp + 2):
                        c0 = cs * 8 + cc
                        u = uch.tile([128, 2, 2, 256], BF16, tag="u")
                        nc.vector.tensor_tensor(
                            out=u, in0=pf[cc],
                            in1=ghw[:, :, c0:c0 + 2].unsqueeze(3)
                            .broadcast_to([128, 2, 2, 256]),
                            op=ALU.mult)
                        ugw[cc] = u
                    # inverse transform restricted to our w-slice, h-major out
                    for ht in (0, 1):
                        pyw = pst([128, 4, 128], "PI", "pyw")
                        for ci in range(4):
                            cc, c2 = ccp + 2 * (ci // 2), ci % 2
                            for kt in (0, 1):
                                nc.tensor.matmul(pyw[:, ci, :],
                                                 lhsT=ugw[cc][:, kt, c2,
                                                              ht * 128:(ht + 1) * 128],
                                                 rhs=tinvw_t[:, kt, :],
                                                 start=(kt == 0), stop=(kt == 1))
                        nc.scalar.activation(ystg2[:, ht, ccp:ccp + 4, :], pyw, AF.Copy)
                for ht in (0, 1):
                    nc.sync.dma_start(
                        out=ysc_d[1, cs * 8:cs * 8 + 8, ht, :, :]
                        .rearrange("c h w -> h c w"),
                        in_=ystg2[:, ht])

            # pool_w = sum_k sigw[k] ghw[k, c] * (T @ xsum_w)[k, c]  -> AllReduce #1
            xsum_wb = gsb.tile([128, 2, 128], BF16, tag="xsumwb")
            nc.vector.tensor_copy(xsum_wb, xsum_w)
            t1 = []
            for kt in (0, 1):
                m1 = pst([128, 128], ("PF", "PI")[kt], "m1")
                for wt in (0, 1):
                    nc.tensor.matmul(m1, lhsT=tfwd_t[:, wt, kt * 128:(kt + 1) * 128],
                                     rhs=xsum_wb[:, wt, :], start=(wt == 0), stop=(wt == 1))
                tt = gsb.tile([128, 128], F32, tag=f"t1_{kt}")
                nc.vector.tensor_tensor(out=tt, in0=m1, in1=ghw[:, kt, :], op=ALU.mult)
                t1.append(tt)
            pw_ps = pst([128, 1], "PF", "pw_ps")
            for kt in (0, 1):
                nc.tensor.matmul(pw_ps, lhsT=t1[kt], rhs=sigw_t[:, kt:kt + 1],
                                 start=(kt == 0), stop=(kt == 1))
            poolw_sb = gsb.tile([128, 1], F32, tag="poolw")
            nc.vector.tensor_copy(poolw_sb, pw_ps)
            nc.sync.dma_start(out=arw_in[:], in_=poolw_sb)
            nc.gpsimd.collective_compute(
                "AllReduce", ALU.add,
                replica_groups=[[0, 1], [2, 3], [4, 5], [6, 7]],
                ins=[arw_in[:]], outs=[arw_out[:]])

            # xsum_h from the resident xh tiles (loads complete during WC),
            # so AllReduce #2 + attention + wsc overlap the HC compute phase
            for cs2 in range(8):
                for ht in (0, 1):
                    nc.vector.tensor_reduce(out=xsum_h[:, ht, cs2 * 16:cs2 * 16 + 16],
                                            in_=xh_ts[cs2][:, ht, :, :],
                                            axis=mybir.AxisListType.X, op=ALU.add)
            # pool_h = ghh[0, :] * sum_{h,w} xh  -> AllReduce #2
            ph_ps = pst([128, 1], "PF", "ph_ps")
            for ht in (0, 1):
                nc.tensor.matmul(ph_ps, lhsT=xsum_h[:, ht, :], rhs=ones_t,
                                 start=(ht == 0), stop=(ht == 1))
            g0_ps = pst([128, 1], "PI", "g0_ps")
            nc.tensor.matmul(g0_ps, lhsT=ghh[0:1, 0, :], rhs=one1_t, start=True, stop=True)
            g0_sb = gsb.tile([128, 1], F32, tag="g0sb")
            nc.vector.tensor_copy(g0_sb, g0_ps)
            poolh_sb = gsb.tile([128, 1], F32, tag="poolh")
            nc.vector.tensor_tensor(out=poolh_sb, in0=ph_ps, in1=g0_sb, op=ALU.mult)
            nc.sync.dma_start(out=arh_in[:], in_=poolh_sb)
            nc.gpsimd.collective_compute(
                "AllReduce", ALU.add,
                replica_groups=[[0, 1], [2, 3], [4, 5], [6, 7]],
                ins=[arh_in[:]], outs=[arh_out[:]])

            # ---------------- HC branch ----------------
            for cs2 in range(8):
                xh_t = xh_ts[cs2]
                ystg2 = stg.tile([128, 2, 16, 128], BF16, tag="ystgh")
                for grp in (0, 8):
                    ccs = (grp, grp + 4)
                    # forward: 2-bank psum per 4-channel group, weight reuse x2
                    pf = {}
                    for cc in ccs:
                        pf[cc] = pst([128, 2, 4, 128], "PF", "pfh")
                    for kt in (0, 1):
                        for ht in (0, 1):
                            for cc in ccs:
                                nc.tensor.matmul(
                                    pf[cc][:, kt],
                                    lhsT=tfwd_t[:, ht, kt * 128:(kt + 1) * 128],
                                    rhs=xh_t[:, ht, cc:cc + 4, :],
                                    start=(ht == 0), stop=(ht == 1))
                    ug = {}
                    for cc in ccs:
                        gc = cs2 * 16 + cc
                        u = uch.tile([128, 2, 4, 128], BF16, tag="u")
                        nc.vector.tensor_tensor(
                            out=u, in0=pf[cc],
                            in1=ghh[:, :, gc:gc + 4].unsqueeze(3)
                            .broadcast_to([128, 2, 4, 128]),
                            op=ALU.mult)
                        ug[cc] = u
                    # inverse: weight reuse x2 across the channel groups
                    for ht in (0, 1):
                        py = {}
                        for cc in ccs:
                            py[cc] = pst([128, 4, 128], "PI", "pyh")
                        for kt in (0, 1):
                            for cc in ccs:
                                nc.tensor.matmul(
                                    py[cc],
                                    lhsT=tinv_t[:, kt, ht * 128:(ht + 1) * 128],
                                    rhs=ug[cc][:, kt],
                                    start=(kt == 0), stop=(kt == 1))
                        for cc in ccs:
                            nc.scalar.activation(ystg2[:, ht, cc:cc + 4, :], py[cc], AF.Copy)
                for ht in (0, 1):
                    nc.sync.dma_start(
                        out=ysc_d[0, cs2 * 16:cs2 * 16 + 16, ht, :, :]
                        .rearrange("c h w -> h c w"),
                        in_=ystg2[:, ht])

            p_sb = []
            for ct, aro in ((0, arh_out), (1, arw_out)):
                pt = gsb.tile([128, 1], F32, tag=f"p_ar{ct}")
                nc.sync.dma_start(out=pt, in_=aro[:])
                p_sb.append(pt)

            # ---------------- channel attention -> folded conv weights ----------------
            q_sb = []
            for ot in (0, 1):
                q_ps = pst([128, 1], ("PF", "PI")[ot], "q_ps")
                for ct in (0, 1):
                    nc.tensor.matmul(q_ps, lhsT=caw1t_t[:, ct, ot * 128:(ot + 1) * 128],
                                     rhs=p_sb[ct], start=(ct == 0), stop=(ct == 1))
                qt = gsb.tile([128, 1], F32, tag=f"q{ot}")
                nc.scalar.activation(qt, q_ps, AF.Gelu, bias=vec_t["cab1"][:, ot:ot + 1])
                nc.vector.tensor_tensor(out=qt, in0=qt, in1=vec_t["dwc"][:, ot:ot + 1],
                                        op=ALU.mult)
                q_sb.append(qt)
            s_sb = []
            for ot in (0, 1):
                s_t = gsb.tile([128, 1], F32, tag=f"s{ot}")
                nc.scalar.activation(s_t, q_sb[ot], AF.Sigmoid, bias=vec_t["dwb"][:, ot:ot + 1])
                s_sb.append(s_t)
            wsc = consts.tile([128, 2, 256], BF16, tag="wsc")
            for ct in (0, 1):
                nc.vector.tensor_scalar_mul(wsc[:, ct, :], lcwt_t[:, ct, :], s_sb[ct])

            # ---------------- conv 1x1 + BN + GELU + residual add ----------------
            for slab in range(16):
                ht, hs = slab // 8, (slab % 8) * 16
                rts = []
                for ct in (0, 1):
                    rt = crhs.tile([128, 16, 128], BF16, tag=f"cr{ct}")
                    nc.scalar.dma_start(out=rt, in_=ysc_d[ct, :, ht, hs:hs + 16, :])
                    rts.append(rt)
                xt = outp.tile([128, 2, 16, 128], BF16, tag="xres")
                nc.sync.dma_start(out=xt, in_=xres_d[slab])
                for ot in (0, 1):
                    gstg = outp.tile([128, 16, 128], BF16, tag=f"gstg{ot}")
                    po = {}
                    for slp in (0, 8):
                        po[slp] = pst([128, 2, 4, 128], "PF", "poc")
                    for ct in (0, 1):
                        for slp in (0, 8):
                            for s2 in (0, 1):
                                nc.tensor.matmul(
                                    po[slp][:, s2],
                                    lhsT=wsc[:, ct, ot * 128:(ot + 1) * 128],
                                    rhs=rts[ct][:, slp + 4 * s2:slp + 4 * s2 + 4, :],
                                    start=(ct == 0), stop=(ct == 1))
                    for slp in (0, 8):
                        nc.scalar.activation(gstg[:, slp:slp + 8, :], po[slp], AF.Gelu,
                                             bias=bnbeff[:, ot:ot + 1],
                                             scale=bninv[:, ot:ot + 1])
                    # residual add (one wide DVE op), then store
                    nc.vector.tensor_add(gstg, gstg, xt[:, ot, :, :])
                    nc.sync.dma_start(out=out_d[slab, :, ot, :, :], in_=gstg)

    nc.compile()
    return nc


_NC_CACHE = None


def _get_nc():
    global _NC_CACHE
    if _NC_CACHE is None:
        _NC_CACHE = _build()
    return _NC_CACHE


def _host_consts(inputs, core):
    """Per-core constant inputs (everything except the x shards)."""
    s = core % 2
    wlo = WS * s
    T = _dft_basis()
    d = {}
    d["tfwd"] = _part_major(np.ascontiguousarray(T.T)).astype(_BF16_NP)
    d["tinv"] = _part_major(T).astype(_BF16_NP)
    d["tinvw"] = _part_major(np.ascontiguousarray(T[:, wlo:wlo + WS])).astype(_BF16_NP)
    d["sigw"] = _part_major(T[:, wlo:wlo + WS].sum(axis=1)).astype(np.float32)
    d["omega"] = (np.arange(129, dtype=np.float32) / 128.0 - 1.0).reshape(1, 129)
    d["lam"] = np.linspace(-1.0, 1.0, 128, dtype=np.float32).reshape(1, 128)
    for m in _MLPS:
        d[f"{m}_w1t"] = np.ascontiguousarray(inputs[f"{m}_w1"].T).astype(np.float32)
        d[f"{m}_b1v"] = inputs[f"{m}_b1"].reshape(64, 1).astype(np.float32)
        d[f"{m}_w2t"] = np.ascontiguousarray(inputs[f"{m}_w2"].T).astype(np.float32)
        d[f"{m}_b2v"] = inputs[f"{m}_b2"].reshape(64, 1).astype(np.float32)
        d[f"{m}_w3t"] = np.ascontiguousarray(inputs[f"{m}_w3"].T).astype(np.float32)
        d[f"{m}_b3v"] = inputs[f"{m}_b3"].reshape(8, 1).astype(np.float32)
    d["caw1t"] = _part_major(np.ascontiguousarray(inputs["ca_w1"].T) / 65536.0).astype(np.float32)
    d["cab1"] = _part_major(inputs["ca_b1"]).astype(np.float32)
    d["dwc"] = _part_major(np.ascontiguousarray(inputs["ca_dw"][:, 1, 1])).astype(np.float32)
    d["dwb"] = _part_major(inputs["ca_db"]).astype(np.float32)
    d["lcwt"] = _part_major(np.ascontiguousarray(inputs["lc_w"].T)).astype(np.float32)
    d["bng"] = _part_major(inputs["bn_g"]).astype(np.float32)
    d["bnb"] = _part_major(inputs["bn_b"]).astype(np.float32)
    d["bnm"] = _part_major(inputs["bn_m"]).astype(np.float32)
    d["bnv"] = _part_major(inputs["bn_v"]).astype(np.float32)
    return d


def kernel(**inputs):
    x = np.asarray(inputs["x"], np.float32)
    nc = _get_nc()

    in_maps = []
    for core in range(NCORES):
        b, s = core // 2, core % 2
        wlo = WS * s
        m = _host_consts(inputs, core)
        xb16 = x[b].astype(_BF16_NP)
        # xh: (8cs2, 128hp, 2ht, 16c, 128w) from x[b, :128, :, wsl]
        m["xh"] = np.ascontiguousarray(
            xb16[:C2, :, wlo:wlo + WS]
            .reshape(8, 16, 2, 128, 128).transpose(0, 3, 2, 1, 4))
        # xw: (16cs, 128wp, 2wt, 8c, 256h) from x[b, 128:, :, :]
        m["xw"] = np.ascontiguousarray(
            xb16[C2:, :, :]
            .reshape(16, 8, 256, 2, 128).transpose(0, 4, 3, 1, 2))
        # xres: (16slab, 128cp, 2ot, 16hh, 128w) from x[b, :, :, wsl]
        m["xres"] = np.ascontiguousarray(
            xb16[:, :, wlo:wlo + WS]
            .reshape(2, 128, 16, 16, 128).transpose(2, 1, 0, 3, 4))
        in_maps.append(m)

    trace = os.environ.get("BASS_KERNEL_TRACE", "0") == "1"
    res = bass_utils.run_bass_kernel_spmd(
        nc, in_maps, core_ids=list(range(NCORES)),
        trace=trace, trace_cores=list(range(NCORES)) if trace else None,
        stitch_traces=False)
    if trace and res.exec_time_ns is not None:
        print(f"HW exec time: {res.exec_time_ns} ns")
        print(f"   mean exec time: {res.mean_exec_time_ns} ns  "
              f"(slowest core {res.max_exec_time_core_id})")
        if res.instructions_and_trace is not None:
            print("   trace:", res.instructions_and_trace[1])

    out = np.empty((B, 2 * C2, N, N), np.float32)
    for core in range(NCORES):
        b, s = core // 2, core % 2
        wlo = WS * s
        o = np.asarray(res.results[core]["out"])  # (16, 128, 2, 16, 128) bf16
        out[b, :, :, wlo:wlo + WS] = (
            o.transpose(2, 1, 0, 3, 4).reshape(256, 256, 128).astype(np.float32))
    return out


# revision 24
# speedup vs baseline: 1.2233x; 1.0613x over previous
"""Trainium2 Bass kernel for the spectral-gating network (nn_DAPSO).

Model (B=4, C=256, H=W=256):
  - channels 0:128   : y_h = irfft(Gh * rfft(x, axis=H))   (per-channel gate)
  - channels 128:256 : y_w = irfft(Gw * rfft(x, axis=W))
  - gates Gh/Gw from tiny MLPs (computed on device)
  - channel attention: s = sigmoid(dw(gelu(W1 @ mean_hw(y) + b)))  -> y *= s
  - y2 = gelu(BN(lc_w @ y));  out = x + y2

Key algorithmic mapping: irfft(G*rfft(x)) along an axis of length N equals
T^T diag(ghat) T x with T the orthonormal real DFT basis (cos/sin rows), so
both branches become dense TensorE matmuls (no FFT).

Sharding: 8 cores = 4 batches x 2 w-halves. Each core computes BOTH branch
outputs for its (batch, w-half) spatial region: the H-branch needs only its
w-columns; the W-branch contracts the full W axis (its forward transform is
duplicated between the pair of cores). The only cross-core communication is
a 1KB AllReduce of the pooled channel means.

Channel attention pooling is computed analytically from input sums (DC
coefficient trick), so it never blocks on branch outputs.

v2 layout/throughput rework vs the original baseline (799us):
  - no f32 residual pre-copy / accumulate-DMA: the conv stage loads x as
    bf16 tiles, adds the residual on GpSimd, and writes a bf16 output
    (host upcasts). Saves ~96 MiB/core of DRAM traffic.
  - all external tensors host-pre-tiled so every DMA is contiguous per
    partition; the branch->conv scratch is c-major so the DMA-transpose
    cost sits on the (PE-bound) branch phase, not the (DMA-bound) conv
    phase.
  - matmul loops restructured for dense PE streams (HAM stays warm).

Per-core external layouts (host-prepped, all bf16 except gate/attn consts):
  xh   (8, 128, 2, 16, 128)  [cs2, hp, ht, c, w]   HC input, w-slice
  xw   (16, 128, 2, 8, 256)  [cs, wp, wt, c, h]    WC input, full w
  xres (16, 128, 2, 16, 128) [slab, cp, ot, hh, w] residual, w-slice
  out  (16, 128, 2, 16, 128) [slab, cp, ot, hh, w] bf16 output
  ysc  (2, 128, 2, 128, 128) [ct, cp, ht, hp, w]   DRAM scratch (bf16)
"""
import sys
import os

sys.path.insert(0, "/opt/trn_rl_repo")

import numpy as np
import ml_dtypes

import concourse.bacc as bacc
import concourse.mybir as mybir
import concourse.tile as tile
from concourse import bass_utils

F32 = mybir.dt.float32
BF16 = mybir.dt.bfloat16
AF = mybir.ActivationFunctionType
ALU = mybir.AluOpType

N = 256          # H = W
C2 = 128         # channels per branch
B = 4
NCORES = 8
WS = 128         # per-core w-slice width

_BF16_NP = ml_dtypes.bfloat16


def _dft_basis():
    """Orthonormal real DFT basis T (N, N): y = T^T diag(ghat) T x == irfft(G*rfft(x))."""
    n = np.arange(N)
    k = np.arange(1, N // 2)
    T = np.zeros((N, N), np.float64)
    T[0, :] = 1.0 / np.sqrt(N)
    T[1:N // 2, :] = np.sqrt(2.0 / N) * np.cos(2 * np.pi * k[:, None] * n[None, :] / N)
    T[N // 2, :] = (1.0 / np.sqrt(N)) * ((-1.0) ** n)
    T[N // 2 + 1:, :] = np.sqrt(2.0 / N) * np.sin(2 * np.pi * k[:, None] * n[None, :] / N)
    return T.astype(np.float32)


def _part_major(a):
    """(256, ...) -> (128, 2, ...) partition-major layout."""
    a = np.asarray(a)
    return np.ascontiguousarray(a.reshape(2, 128, *a.shape[1:]).transpose(
        (1, 0) + tuple(range(2, a.ndim + 1))))


_MLPS = ("ah", "bc1", "aw", "bc2")


def _build():
    nc = bacc.Bacc("TRN2", target_bir_lowering=False, num_devices=NCORES)

    # ---------------- I/O declarations ----------------
    xh_d = nc.dram_tensor("xh", [8, 128, 2, 16, 128], BF16, kind="ExternalInput")
    xw_d = nc.dram_tensor("xw", [16, 128, 2, 8, 256], BF16, kind="ExternalInput")
    xres_d = nc.dram_tensor("xres", [16, 128, 2, 16, 128], BF16, kind="ExternalInput")
    pk1_d = nc.dram_tensor("pk1", [1, 513], F32, kind="ExternalInput")
    pk64_d = nc.dram_tensor("pk64", [64, 300], F32, kind="ExternalInput")
    vpack_d = nc.dram_tensor("vpack", [128, 18], F32, kind="ExternalInput")
    tfwd_d = nc.dram_tensor("tfwd", [128, 2, 256], BF16, kind="ExternalInput")
    tinv_d = nc.dram_tensor("tinv", [128, 2, 256], BF16, kind="ExternalInput")
    tinvw_d = nc.dram_tensor("tinvw", [128, 2, 128], BF16, kind="ExternalInput")
    caw1t_d = nc.dram_tensor("caw1t", [128, 2, 256], F32, kind="ExternalInput")
    lcwt_d = nc.dram_tensor("lcwt", [128, 2, 256], F32, kind="ExternalInput")

    out_d = nc.dram_tensor("out", [16, 128, 2, 16, 128], BF16, kind="ExternalOutput")

    ysc_d = nc.dram_tensor("ysc", [2, 128, 2, 128, 128], BF16)  # (ct, cp, ht, hp, w)
    arh_in = nc.dram_tensor("arh_in", [128, 1], F32)
    arh_out = nc.dram_tensor("arh_out", [128, 1], F32)
    arw_in = nc.dram_tensor("arw_in", [128, 1], F32)
    arw_out = nc.dram_tensor("arw_out", [128, 1], F32)

    with tile.TileContext(nc) as tc:
        with tc.tile_pool(name="consts", bufs=1) as consts, \
             tc.tile_pool(name="xin", bufs=3) as xin, \
             tc.tile_pool(name="xhp", bufs=8) as xhp, \
             tc.tile_pool(name="uch", bufs=3) as uch, \
             tc.tile_pool(name="stg", bufs=2) as stg, \
             tc.tile_pool(name="crhs", bufs=2) as crhs, \
             tc.tile_pool(name="outp", bufs=2) as outp, \
             tc.tile_pool(name="gsb", bufs=1) as gsb, \
             tc.tile_pool(name="ps", bufs=2, space="PSUM") as ps:

            def pst(shape, tag, name):
                return ps.tile(shape, F32, tag=tag, bufs=3 if tag == "PF" else 2,
                               name=name)

            # ------- first WC input loads (kick these off before anything) ----
            xw_t_pre = []
            for cs in range(2):
                xt = xin.tile([128, 2, 8, 256], BF16, tag="xw")
                nc.sync.dma_start(out=xt, in_=xw_d[cs])
                xw_t_pre.append(xt)

            # ---------------- const loads (scalar HWDGE queue) ----------------
            # tiny gate-MLP consts packed into 3 DMAs (each dma_start costs
            # ~2us fixed; 30 separate loads would stall the gate chain)
            pk1_t = consts.tile([1, 513], F32, tag="pk1")
            nc.scalar.dma_start(out=pk1_t, in_=pk1_d[:])
            pk64_t = consts.tile([64, 300], F32, tag="pk64")
            nc.scalar.dma_start(out=pk64_t, in_=pk64_d[:])
            vpack_t = consts.tile([128, 18], F32, tag="vpack")
            nc.scalar.dma_start(out=vpack_t, in_=vpack_d[:])
            omega_t = pk1_t[:, 0:129]
            lam_t = pk1_t[:, 129:257]
            mlp_t = {}
            for mi, m in enumerate(_MLPS):
                mlp_t[(m, "w1t")] = pk1_t[:, 257 + mi * 64:257 + (mi + 1) * 64]
                base = mi * 75
                mlp_t[(m, "b1")] = pk64_t[:, base:base + 1]
                mlp_t[(m, "w2t")] = pk64_t[:, base + 1:base + 65]
                mlp_t[(m, "b2")] = pk64_t[:, base + 65:base + 66]
                mlp_t[(m, "w3t")] = pk64_t[:, base + 66:base + 74]
                mlp_t[(m, "b3")] = pk64_t[0:8, base + 74:base + 75]
            vec_t = {}
            for vi, nm in enumerate(("cab1", "dwc", "dwb", "bng", "bnb", "bnm", "bnv")):
                vec_t[nm] = vpack_t[:, 2 * vi:2 * vi + 2]
            sigw_t = vpack_t[:, 14:16]
            tfwd_t = consts.tile([128, 2, 256], BF16, tag="tfwd")
            nc.scalar.dma_start(out=tfwd_t, in_=tfwd_d[:])
            tinvw_t = consts.tile([128, 2, 128], BF16, tag="tinvw")
            nc.scalar.dma_start(out=tinvw_t, in_=tinvw_d[:])
            tinv_t = consts.tile([128, 2, 256], BF16, tag="tinv")
            nc.scalar.dma_start(out=tinv_t, in_=tinv_d[:])
            caw1t_t = consts.tile([128, 2, 256], F32, tag="caw1t")
            nc.scalar.dma_start(out=caw1t_t, in_=caw1t_d[:])
            lcwt_t = consts.tile([128, 2, 256], F32, tag="lcwt")
            nc.scalar.dma_start(out=lcwt_t, in_=lcwt_d[:])
            ones_t = consts.tile([128, 1], F32, tag="ones")
            nc.vector.memset(ones_t, 1.0)
            one1_t = consts.tile([1, 1], F32, tag="one1")
            nc.vector.memset(one1_t, 1.0)
            # full HC input residency (8 MiB): streams on the scalar queue
            # during the WC phase so the HC phase never waits on loads
            xh_ts = []
            for cs2 in range(8):
                xt = xhp.tile([128, 2, 16, 128], BF16, tag="xh")
                nc.scalar.dma_start(out=xt, in_=xh_d[cs2])
                xh_ts.append(xt)

            # ---------------- gate MLPs (tiny), table-batched ----------------
            def mlp_head(m, xvec, nk, role):
                w1t, b1 = mlp_t[(m, "w1t")], mlp_t[(m, "b1")]
                w2t, b2 = mlp_t[(m, "w2t")], mlp_t[(m, "b2")]
                w3t, b3 = mlp_t[(m, "w3t")], mlp_t[(m, "b3")]

                p1 = pst([64, nk], "PF", "p1")
                nc.tensor.matmul(p1, lhsT=w1t, rhs=xvec, start=True, stop=True)
                h1 = gsb.tile([64, nk], F32, tag="m_h1")
                nc.scalar.activation(h1, p1, AF.Gelu, bias=b1)
                p2 = pst([64, nk], "PI", "p2")
                nc.tensor.matmul(p2, lhsT=w2t, rhs=h1, start=True, stop=True)
                h2 = gsb.tile([64, nk], F32, tag="m_h2")
                nc.scalar.activation(h2, p2, AF.Gelu, bias=b2)
                p3 = pst([8, nk], "PF", "p3")
                nc.tensor.matmul(p3, lhsT=w3t, rhs=h2, start=True, stop=True)
                at = gsb.tile([8, nk], F32, tag=f"m_at{role}")
                nc.scalar.activation(at, p3, AF.Identity, bias=b3)
                return at

            ghh = consts.tile([128, 2, 128], F32, tag="ghh")
            ghw = consts.tile([128, 2, 128], F32, tag="ghw")
            gp = {}
            gtags = {("h", 0): "PF", ("h", 1): "PI", ("w", 0): "PF", ("w", 1): "PI"}
            for (am, bm, nmk) in (("aw", "bc2", "w"), ("ah", "bc1", "h")):
                at = mlp_head(am, omega_t, 129, "a")
                bt = mlp_head(bm, lam_t, 128, "b")
                g0 = pst([128, 128], gtags[(nmk, 0)], "g0")
                nc.tensor.matmul(g0, lhsT=at[:, 0:128], rhs=bt, start=True, stop=True)
                gn = pst([1, 128], gtags[(nmk, 1)], "gn")
                nc.tensor.matmul(gn, lhsT=at[:, 128:129], rhs=bt, start=True, stop=True)
                gp[(nmk, 0)] = g0
                gp[(nmk, 1)] = gn
            # softplus(z) = relu(z) + log1p(exp(-|z|)), stage-batched across all 4
            keys = list(gp.keys())
            sp = {}
            for i, key in enumerate(keys):
                npart = 128 if key[1] == 0 else 1
                na = gsb.tile([128, 128], F32, tag=f"sp_na{i}")
                nc.scalar.activation(na[:npart, :], gp[key], AF.Abs)
                sp[key] = na
            for i, key in enumerate(keys):
                npart = 128 if key[1] == 0 else 1
                ex = gsb.tile([128, 128], F32, tag=f"sp_ex{i}")
                nc.scalar.activation(ex[:npart, :], sp[key][:npart, :], AF.Exp, scale=-1.0)
                nc.vector.tensor_scalar_add(ex[:npart, :], ex[:npart, :], 1.0)
                sp[key] = ex
            for key in keys:
                npart = 128 if key[1] == 0 else 1
                nc.scalar.activation(sp[key][:npart, :], sp[key][:npart, :], AF.Ln)
            for i, key in enumerate(keys):
                npart = 128 if key[1] == 0 else 1
                re = gsb.tile([128, 128], F32, tag=f"sp_re{i}")
                nc.scalar.activation(re[:npart, :], gp[key], AF.Relu)
                gh = ghh if key[0] == "h" else ghw
                if key[1] == 0:
                    nc.vector.tensor_add(gh[:, 0, :], sp[key][:128, :], re[:128, :])
                else:
                    # rows 128+j of ghat equal G[j]: copy the aligned block first,
                    # then overwrite row 0 with the Nyquist G[128].
                    nc.vector.tensor_copy(gh[:, 1, :], gh[:, 0, :])
                    nc.vector.tensor_add(gh[0:1, 1, :], sp[key][0:1, :], re[0:1, :])
            for gh in (ghh, ghw):
                nc.vector.tensor_scalar_mul(gh[:, :, :], gh[:, :, :], float(8.0 ** -0.5))

            # ---------------- BN prep ----------------
            bninv = consts.tile([128, 2], F32, tag="bninv")
            nc.vector.tensor_scalar_add(bninv, vec_t["bnv"], 1e-5)
            nc.scalar.activation(bninv, bninv, AF.Sqrt)
            nc.vector.reciprocal(bninv, bninv)
            nc.vector.tensor_tensor(out=bninv, in0=vec_t["bng"], in1=bninv, op=ALU.mult)
            bnbeff = consts.tile([128, 2], F32, tag="bnbeff")
            nc.vector.tensor_tensor(out=bnbeff, in0=vec_t["bnm"], in1=bninv, op=ALU.mult)
            nc.vector.tensor_tensor(out=bnbeff, in0=vec_t["bnb"], in1=bnbeff, op=ALU.subtract)

            xsum_h = consts.tile([128, 2, 128], F32, tag="xsumh")      # [h, ht, c]
            xsum_w = consts.tile([128, 2, 128], F32, tag="xsumw")      # [w, wt, c]

            # ---------------- WC branch (first: its pooled sums gate the AR) ----
            for cs in range(16):
                if cs < 2:
                    xw_t = xw_t_pre[cs]
                else:
                    xw_t = xin.tile([128, 2, 8, 256], BF16, tag="xw")
                    nc.sync.dma_start(out=xw_t, in_=xw_d[cs])
                # h-sum: two GPSIMD tree levels (frees the xw slot fast), DVE reduce
                for wt in (0, 1):
                    tr1 = gsb.tile([128, 8, 128], BF16, tag=f"tree1{wt}")
                    nc.gpsimd.tensor_add(tr1, xw_t[:, wt, :, 0:128], xw_t[:, wt, :, 128:256])
                    tr2 = gsb.tile([128, 8, 64], BF16, tag=f"tree2{wt}")
                    nc.gpsimd.tensor_add(tr2, tr1[:, :, 0:64], tr1[:, :, 64:128])
                    nc.vector.tensor_reduce(out=xsum_w[:, wt, cs * 8:cs * 8 + 8], in_=tr2,
                                            axis=mybir.AxisListType.X, op=ALU.add)
                # spread the HC pooling reduces over the WC phase (DVE)
                if cs >= 8:
                    for ht in (0, 1):
                        nc.vector.tensor_reduce(
                            out=xsum_h[:, ht, (cs - 8) * 16:(cs - 8) * 16 + 16],
                            in_=xh_ts[cs - 8][:, ht, :, :],
                            axis=mybir.AxisListType.X, op=ALU.add)
                ystg2 = stg.tile([128, 2, 8, 128], BF16, tag="ystgw")
                for ccp in (0, 4):
                    # forward transform: u[k, c, h]; 2-bank psum per 2-channel
                    # group, weight-reuse order (each lhsT serves 2 matmuls)
                    pf = {}
                    for cc in (ccp, ccp + 2):
                        pf[cc] = pst([128, 2, 2, 256], "PF", "pfw")
                    for kt in (0, 1):
                        for wt in (0, 1):
                            for cc in (ccp, ccp + 2):
                                nc.tensor.matmul(
                                    pf[cc][:, kt],
                                    lhsT=tfwd_t[:, wt, kt * 128:(kt + 1) * 128],
                                    rhs=xw_t[:, wt, cc:cc + 2, :],
                                    start=(wt == 0), stop=(wt == 1))
                    ugw = {}
                    for cc in (ccp, ccp + 2):
                        c0 = cs * 8 + cc
                        u = uch.tile([128, 2, 2, 256], BF16, tag="u")
                        nc.vector.tensor_tensor(
                            out=u[:, 0], in0=pf[cc][:, 0],
                            in1=ghw[:, 0, c0:c0 + 2].unsqueeze(2)
                            .broadcast_to([128, 2, 256]),
                            op=ALU.mult)
                        for c2 in (0, 1):
                            nc.scalar.activation(
                                u[:, 1, c2, :], pf[cc][:, 1, c2, :], AF.Copy,
                                scale=ghw[:, 1, c0 + c2:c0 + c2 + 1])
                        ugw[cc] = u
                    # inverse transform restricted to our w-slice, h-major out
                    for ht in (0, 1):
                        pyw = pst([128, 4, 128], "PI", "pyw")
                        for ci in range(4):
                            cc, c2 = ccp + 2 * (ci // 2), ci % 2
                            for kt in (0, 1):
                                nc.tensor.matmul(pyw[:, ci, :],
                                                 lhsT=ugw[cc][:, kt, c2,
                                                              ht * 128:(ht + 1) * 128],
                                                 rhs=tinvw_t[:, kt, :],
                                                 start=(kt == 0), stop=(kt == 1))
                        nc.scalar.activation(ystg2[:, ht, ccp:ccp + 4, :], pyw, AF.Copy)
                for ht in (0, 1):
                    nc.sync.dma_start(
                        out=ysc_d[1, cs * 8:cs * 8 + 8, ht, :, :]
                        .rearrange("c h w -> h c w"),
                        in_=ystg2[:, ht])

            # pool_w = sum_k sigw[k] ghw[k, c] * (T @ xsum_w)[k, c]  -> AllReduce #1
            xsum_wb = gsb.tile([128, 2, 128], BF16, tag="xsumwb")
            nc.vector.tensor_copy(xsum_wb, xsum_w)
            t1 = []
            for kt in (0, 1):
                m1 = pst([128, 128], ("PF", "PI")[kt], "m1")
                for wt in (0, 1):
                    nc.tensor.matmul(m1, lhsT=tfwd_t[:, wt, kt * 128:(kt + 1) * 128],
                                     rhs=xsum_wb[:, wt, :], start=(wt == 0), stop=(wt == 1))
                tt = gsb.tile([128, 128], F32, tag=f"t1_{kt}")
                nc.vector.tensor_tensor(out=tt, in0=m1, in1=ghw[:, kt, :], op=ALU.mult)
                t1.append(tt)
            pw_ps = pst([128, 1], "PF", "pw_ps")
            for kt in (0, 1):
                nc.tensor.matmul(pw_ps, lhsT=t1[kt], rhs=sigw_t[:, kt:kt + 1],
                                 start=(kt == 0), stop=(kt == 1))
            poolw_sb = gsb.tile([128, 1], F32, tag="poolw")
            nc.vector.tensor_copy(poolw_sb, pw_ps)
            nc.sync.dma_start(out=arw_in[:], in_=poolw_sb)
            nc.gpsimd.collective_compute(
                "AllReduce", ALU.add,
                replica_groups=[[0, 1], [2, 3], [4, 5], [6, 7]],
                ins=[arw_in[:]], outs=[arw_out[:]])

            # pool_h = ghh[0, :] * sum_{h,w} xh  -> AllReduce #2
            ph_ps = pst([128, 1], "PF", "ph_ps")
            for ht in (0, 1):
                nc.tensor.matmul(ph_ps, lhsT=xsum_h[:, ht, :], rhs=ones_t,
                                 start=(ht == 0), stop=(ht == 1))
            g0_ps = pst([128, 1], "PI", "g0_ps")
            nc.tensor.matmul(g0_ps, lhsT=ghh[0:1, 0, :], rhs=one1_t, start=True, stop=True)
            g0_sb = gsb.tile([128, 1], F32, tag="g0sb")
            nc.vector.tensor_copy(g0_sb, g0_ps)
            poolh_sb = gsb.tile([128, 1], F32, tag="poolh")
            nc.vector.tensor_tensor(out=poolh_sb, in0=ph_ps, in1=g0_sb, op=ALU.mult)
            nc.sync.dma_start(out=arh_in[:], in_=poolh_sb)
            nc.gpsimd.collective_compute(
                "AllReduce", ALU.add,
                replica_groups=[[0, 1], [2, 3], [4, 5], [6, 7]],
                ins=[arh_in[:]], outs=[arh_out[:]])

            # ---------------- HC branch ----------------
            for cs2 in range(8):
                xh_t = xh_ts[cs2]
                ystg2 = stg.tile([128, 2, 16, 128], BF16, tag="ystgh")
                for grp in (0, 8):
                    ccs = (grp, grp + 4)
                    # forward: 2-bank psum per 4-channel group, weight reuse x2
                    pf = {}
                    for cc in ccs:
                        pf[cc] = pst([128, 2, 4, 128], "PF", "pfh")
                    for kt in (0, 1):
                        for ht in (0, 1):
                            for cc in ccs:
                                nc.tensor.matmul(
                                    pf[cc][:, kt],
                                    lhsT=tfwd_t[:, ht, kt * 128:(kt + 1) * 128],
                                    rhs=xh_t[:, ht, cc:cc + 4, :],
                                    start=(ht == 0), stop=(ht == 1))
                    ug = {}
                    for cc in ccs:
                        gc = cs2 * 16 + cc
                        u = uch.tile([128, 2, 4, 128], BF16, tag="u")
                        nc.vector.tensor_tensor(
                            out=u, in0=pf[cc],
                            in1=ghh[:, :, gc:gc + 4].unsqueeze(3)
                            .broadcast_to([128, 2, 4, 128]),
                            op=ALU.mult)
                        ug[cc] = u
                    # inverse: weight reuse x2 across the channel groups
                    for ht in (0, 1):
                        py = {}
                        for cc in ccs:
                            py[cc] = pst([128, 4, 128], "PI", "pyh")
                        for kt in (0, 1):
                            for cc in ccs:
                                nc.tensor.matmul(
                                    py[cc],
                                    lhsT=tinv_t[:, kt, ht * 128:(ht + 1) * 128],
                                    rhs=ug[cc][:, kt],
                                    start=(kt == 0), stop=(kt == 1))
                        for cc in ccs:
                            nc.scalar.activation(ystg2[:, ht, cc:cc + 4, :], py[cc], AF.Copy)
                for ht in (0, 1):
                    nc.sync.dma_start(
                        out=ysc_d[0, cs2 * 16:cs2 * 16 + 16, ht, :, :]
                        .rearrange("c h w -> h c w"),
                        in_=ystg2[:, ht])

            p_sb = []
            for ct, aro in ((0, arh_out), (1, arw_out)):
                pt = gsb.tile([128, 1], F32, tag=f"p_ar{ct}")
                nc.sync.dma_start(out=pt, in_=aro[:])
                p_sb.append(pt)

            # ---------------- channel attention -> folded conv weights ----------------
            q_sb = []
            for ot in (0, 1):
                q_ps = pst([128, 1], ("PF", "PI")[ot], "q_ps")
                for ct in (0, 1):
                    nc.tensor.matmul(q_ps, lhsT=caw1t_t[:, ct, ot * 128:(ot + 1) * 128],
                                     rhs=p_sb[ct], start=(ct == 0), stop=(ct == 1))
                qt = gsb.tile([128, 1], F32, tag=f"q{ot}")
                nc.scalar.activation(qt, q_ps, AF.Gelu, bias=vec_t["cab1"][:, ot:ot + 1])
                nc.vector.tensor_tensor(out=qt, in0=qt, in1=vec_t["dwc"][:, ot:ot + 1],
                                        op=ALU.mult)
                q_sb.append(qt)
            s_sb = []
            for ot in (0, 1):
                s_t = gsb.tile([128, 1], F32, tag=f"s{ot}")
                nc.scalar.activation(s_t, q_sb[ot], AF.Sigmoid, bias=vec_t["dwb"][:, ot:ot + 1])
                s_sb.append(s_t)
            wsc = consts.tile([128, 2, 256], BF16, tag="wsc")
            for ct in (0, 1):
                nc.vector.tensor_scalar_mul(wsc[:, ct, :], lcwt_t[:, ct, :], s_sb[ct])

            # ---------------- conv 1x1 + BN + GELU + residual add ----------------
            # residual tiles prefetch 4 slabs deep (xres has no ysc dependency,
            # so these loads run during the attention/AllReduce latency)
            xres_ts = {}
            for s in range(3):
                xt = outp.tile([128, 2, 16, 128], BF16, tag="xres", bufs=3, name="xt")
                nc.sync.dma_start(out=xt, in_=xres_d[s])
                xres_ts[s] = xt
            for slab in range(16):
                ht, hs = slab // 8, (slab % 8) * 16
                rts = []
                for ct in (0, 1):
                    rt = crhs.tile([128, 16, 128], BF16, tag=f"cr{ct}")
                    nc.scalar.dma_start(out=rt, in_=ysc_d[ct, :, ht, hs:hs + 16, :])
                    rts.append(rt)
                if slab < 3:
                    xt = xres_ts[slab]
                else:
                    xt = outp.tile([128, 2, 16, 128], BF16, tag="xres", bufs=3, name="xt")
                    nc.sync.dma_start(out=xt, in_=xres_d[slab])
                for ot in (0, 1):
                    gstg = outp.tile([128, 16, 128], BF16, tag=f"gstg{ot}")
                    po = {}
                    for slp in (0, 8):
                        po[slp] = pst([128, 2, 4, 128], "PF", "poc")
                    for ct in (0, 1):
                        for slp in (0, 8):
                            for s2 in (0, 1):
                                nc.tensor.matmul(
                                    po[slp][:, s2],
                                    lhsT=wsc[:, ct, ot * 128:(ot + 1) * 128],
                                    rhs=rts[ct][:, slp + 4 * s2:slp + 4 * s2 + 4, :],
                                    start=(ct == 0), stop=(ct == 1))
                    for slp in (0, 8):
                        nc.scalar.activation(gstg[:, slp:slp + 8, :], po[slp], AF.Gelu,
                                             bias=bnbeff[:, ot:ot + 1],
                                             scale=bninv[:, ot:ot + 1])
                    # residual add (one wide DVE op), then store
                    nc.vector.tensor_add(gstg, gstg, xt[:, ot, :, :])
                    nc.sync.dma_start(out=out_d[slab, :, ot, :, :], in_=gstg)

    nc.compile()
    return nc


_NC_CACHE = None


def _get_nc():
    global _NC_CACHE
    if _NC_CACHE is None:
        _NC_CACHE = _build()
    return _NC_CACHE


def _host_consts(inputs, core):
    """Per-core constant inputs (everything except the x shards)."""
    s = core % 2
    wlo = WS * s
    T = _dft_basis()
    d = {}
    d["tfwd"] = _part_major(np.ascontiguousarray(T.T)).astype(_BF16_NP)
    d["tinv"] = _part_major(T).astype(_BF16_NP)
    d["tinvw"] = _part_major(np.ascontiguousarray(T[:, wlo:wlo + WS])).astype(_BF16_NP)
    pk1 = np.zeros((1, 513), np.float32)
    pk1[0, 0:129] = np.arange(129, dtype=np.float32) / 128.0 - 1.0
    pk1[0, 129:257] = np.linspace(-1.0, 1.0, 128, dtype=np.float32)
    pk64 = np.zeros((64, 300), np.float32)
    for mi, m in enumerate(_MLPS):
        pk1[0, 257 + mi * 64:257 + (mi + 1) * 64] = inputs[f"{m}_w1"].reshape(64)
        base = mi * 75
        pk64[:, base] = inputs[f"{m}_b1"]
        pk64[:, base + 1:base + 65] = np.asarray(inputs[f"{m}_w2"]).T
        pk64[:, base + 65] = inputs[f"{m}_b2"]
        pk64[:, base + 66:base + 74] = np.asarray(inputs[f"{m}_w3"]).T
        pk64[0:8, base + 74] = inputs[f"{m}_b3"]
    d["pk1"] = pk1
    d["pk64"] = pk64
    vpack = np.zeros((128, 18), np.float32)
    for vi, nm in enumerate(("ca_b1", None, "ca_db", "bn_g", "bn_b", "bn_m", "bn_v")):
        if nm is None:
            vpack[:, 2 * vi:2 * vi + 2] = _part_major(
                np.ascontiguousarray(inputs["ca_dw"][:, 1, 1]))
        else:
            vpack[:, 2 * vi:2 * vi + 2] = _part_major(inputs[nm])
    vpack[:, 14:16] = _part_major(T[:, wlo:wlo + WS].sum(axis=1))
    d["vpack"] = vpack
    d["caw1t"] = _part_major(np.ascontiguousarray(inputs["ca_w1"].T) / 65536.0).astype(np.float32)
    d["lcwt"] = _part_major(np.ascontiguousarray(inputs["lc_w"].T)).astype(np.float32)
    return d


def kernel(**inputs):
    x = np.asarray(inputs["x"], np.float32)
    nc = _get_nc()

    in_maps = []
    for core in range(NCORES):
        b, s = core // 2, core % 2
        wlo = WS * s
        m = _host_consts(inputs, core)
        xb16 = x[b].astype(_BF16_NP)
        # xh: (8cs2, 128hp, 2ht, 16c, 128w) from x[b, :128, :, wsl]
        m["xh"] = np.ascontiguousarray(
            xb16[:C2, :, wlo:wlo + WS]
            .reshape(8, 16, 2, 128, 128).transpose(0, 3, 2, 1, 4))
        # xw: (16cs, 128wp, 2wt, 8c, 256h) from x[b, 128:, :, :]
        m["xw"] = np.ascontiguousarray(
            xb16[C2:, :, :]
            .reshape(16, 8, 256, 2, 128).transpose(0, 4, 3, 1, 2))
        # xres: (16slab, 128cp, 2ot, 16hh, 128w) from x[b, :, :, wsl]
        m["xres"] = np.ascontiguousarray(
            xb16[:, :, wlo:wlo + WS]
            .reshape(2, 128, 16, 16, 128).transpose(2, 1, 0, 3, 4))
        in_maps.append(m)

    trace = os.environ.get("BASS_KERNEL_TRACE", "0") == "1"
    res = bass_utils.run_bass_kernel_spmd(
        nc, in_maps, core_ids=list(range(NCORES)),
        trace=trace, trace_cores=list(range(NCORES)) if trace else None,
        stitch_traces=False)
    if trace and res.exec_time_ns is not None:
        print(f"HW exec time: {res.exec_time_ns} ns")
        print(f"   mean exec time: {res.mean_exec_time_ns} ns  "
              f"(slowest core {res.max_exec_time_core_id})")
        if res.instructions_and_trace is not None:
            print("   trace:", res.instructions_and_trace[1])

    out = np.empty((B, 2 * C2, N, N), np.float32)
    for core in range(NCORES):
        b, s = core // 2, core % 2
        wlo = WS * s
        o = np.asarray(res.results[core]["out"])  # (16, 128, 2, 16, 128) bf16
        out[b, :, :, wlo:wlo + WS] = (
            o.transpose(2, 1, 0, 3, 4).reshape(256, 256, 128).astype(np.float32))
    return out
